# revision 1
# baseline (speedup 1.0000x reference)
"""Causal multi-head attention (B=1, S=4096, D=1024, H=16, HD=64) on 8 TRN2
NeuronCores.

Sharding: tensor-parallel over heads. Core c owns heads [2c, 2c+1]: Wq/Wk/Wv
column slices (128 cols) and Wo row slice (128 rows). Each core computes its
partial output projection over the full sequence; the host sums the 8 partials
and adds bo (the row-parallel all-reduce done at gather time).

Device kernel layout (per core; matmul operands in bf16, fp32 PSUM accum,
flip USE_BF16 off for float32r if tighter accuracy is ever needed):
  - x is fed pre-transposed (xT [D, S]) so the QKV projections need no
    on-chip transpose: Q^T/K^T/V^T [c=128(2 heads x 64), s] = W.T @ xT.
  - scores are computed TRANSPOSED: scT[sk, sq] = K_h @ Q_h^T per 128-row
    sk-tile (the two heads' K=64 matmuls run concurrently via PE row
    tiling), which makes exp(scores) directly usable as the stationary
    operand of attn@V -- no P transposes. Softmax denominators come for
    free from a ones-column appended to V (row 64 of the attn@V
    accumulator); the 1/sqrt(HD) scale is folded into Wq/bq on the host;
    max-subtraction is skipped (scores ~ N(0,1), exp is safe in fp32).
  - V^T -> V via PE transposes of [128,128] blocks.
  - causal masking: one DVE multiply per diagonal tile against a
    host-provided mask that is zero below the 128-aligned diagonal band
    and upper-triangular on it.
  - rescale by the denominator reciprocal (fast DVE recip + gpsimd
    partition-broadcast), applied to attn-out^T before the output
    projection (the divide commutes through the linear projection).
  - the attn@V matmuls are software-pipelined one sk-tile behind the
    scores/exp so the in-order PE queue never waits on the ACT engine,
    and each chunk's output projection is deferred one chunk so the
    rescale latency chain is hidden behind the next chunk's QKV matmuls.
"""
import sys

sys.path.insert(0, "/opt/trn_rl_repo")

import numpy as np

import concourse.bacc as bacc
import concourse.mybir as mybir
import concourse.tile as tile
from concourse.bass_utils import run_bass_kernel_spmd
from concourse.dve_ops import RECIPROCAL_APPROX_FAST, RECIP_APPROX_FAST_CONSTS

F32 = mybir.dt.float32
F32R = mybir.dt.float32r
BF16 = mybir.dt.bfloat16
USE_BF16 = True
CDT = BF16 if USE_BF16 else F32R  # matmul-operand compute dtype
EXP = mybir.ActivationFunctionType.Exp

S, D, H, HD = 4096, 1024, 16, 64
NCORES = 8
CPC = D // NCORES  # 128 head-dim columns per core (2 heads)
NS = S // 512      # 8 chunks of 512 along the sequence
ND = D // 128      # 8 contraction chunks for the projections


def _build_nc(dbg=False):
    nc = bacc.Bacc("TRN2", target_bir_lowering=False, debug=False,
                   num_devices=NCORES)
    xT = nc.dram_tensor("xT", [D, S], CDT, kind="ExternalInput").ap()
    w3 = nc.dram_tensor("w3", [128, 3, ND, 128], CDT, kind="ExternalInput").ap()
    wo = nc.dram_tensor("wo", [128, D], CDT, kind="ExternalInput").ap()
    bq = nc.dram_tensor("bq", [CPC, 1], F32, kind="ExternalInput").ap()
    bk = nc.dram_tensor("bk", [CPC, 1], F32, kind="ExternalInput").ap()
    bv = nc.dram_tensor("bv", [CPC, 1], F32, kind="ExternalInput").ap()
    tri = nc.dram_tensor("tri", [128, 4, 512], CDT, kind="ExternalInput").ap()
    ident = nc.dram_tensor("ident", [128, 128], F32, kind="ExternalInput").ap()
    out = nc.dram_tensor("out", [S, D], F32, kind="ExternalOutput").ap()
    dbg_t = {}
    if dbg:
        for name, shape in (("d_QT", [128, S]), ("d_KT", [128, S]),
                            ("d_VP", [128, 2, 32, 65]), ("d_OT0", [64, S]),
                            ("d_OT1", [64, S]), ("d_pt", [128, 8, 1024]),
                            ("d_ot0", [65, 512]), ("d_ot1", [65, 512])):
            dbg_t[name] = nc.dram_tensor(name, shape, F32R,
                                         kind="ExternalOutput").ap()
        for name, shape in (("d_rec", [2, 512]), ("d_bc", [64, 512])):
            dbg_t[name] = nc.dram_tensor(name, shape, F32,
                                         kind="ExternalOutput").ap()

    with tile.TileContext(nc) as tc:
        _emit(nc, tc, xT, w3, wo, bq, bk, bv, tri, ident, out, dbg_t)
    nc.compile()
    return nc


def _emit(nc, tc, xT, w3, wo, bq, bk, bv, tri, ident, out, dbg_t=None):
    from contextlib import ExitStack
    ctx = ExitStack()
    with ctx:
        consts = ctx.enter_context(tc.tile_pool(name="consts", bufs=1))
        persist = ctx.enter_context(tc.tile_pool(name="persist", bufs=1))
        xt_pool = ctx.enter_context(tc.tile_pool(name="xt", bufs=4))
        vt_pool = ctx.enter_context(tc.tile_pool(name="vt", bufs=4))
        pt_pool = ctx.enter_context(tc.tile_pool(name="pt", bufs=12))
        rec_pool = ctx.enter_context(tc.tile_pool(name="rec", bufs=4))
        bc_pool = ctx.enter_context(tc.tile_pool(name="bc", bufs=4))
        ost_pool = ctx.enter_context(tc.tile_pool(name="ost", bufs=8))
        ps_mm = ctx.enter_context(tc.tile_pool(name="psmm", bufs=2, space="PSUM"))
        ps_sc = ctx.enter_context(tc.tile_pool(name="pssc", bufs=2, space="PSUM"))
        ps_ot = ctx.enter_context(tc.tile_pool(name="psot", bufs=2, space="PSUM"))

        # ---- constants (wq slice first so the first matmul starts early;
        # the rest after the first chunk's xt loads) ----------------------
        w_sb = consts.tile([128, 3, ND, 128], CDT)
        nc.sync.dma_start(out=w_sb[:, 0], in_=w3[:, 0])
        bq_sb = consts.tile([CPC, 1], F32)
        bk_sb = consts.tile([CPC, 1], F32)
        bv_sb = consts.tile([CPC, 1], F32)
        nc.sync.dma_start(out=bq_sb, in_=bq)
        nc.sync.dma_start(out=bk_sb, in_=bk)
        nc.sync.dma_start(out=bv_sb, in_=bv)
        id_sb = consts.tile([128, 128], F32)
        tri_sb = consts.tile([128, 4, 512], CDT)
        wo_sb = consts.tile([128, D], CDT)

        def emit_late_consts():
            nc.sync.dma_start(out=w_sb[:, 1], in_=w3[:, 1])
            nc.sync.dma_start(out=w_sb[:, 2], in_=w3[:, 2])
            nc.sync.dma_start(out=tri_sb, in_=tri)
            nc.sync.dma_start(out=id_sb, in_=ident)
            nc.sync.dma_start(out=wo_sb, in_=wo)

        # ---- persistent activations -----------------------------------
        QT = persist.tile([128, S], CDT)      # [c(2 heads x 64), s]
        KT = persist.tile([128, S], CDT)
        VP = persist.tile([128, 2, 32, 65], CDT)  # V natural + ones col, per (h, sk-tile)
        OT = persist.tile([128, S], CDT)      # rescaled attn out^T, both heads

        from collections import deque
        filler = deque()  # deferred projection units, drained 1-per-j-iter

        def push_proj(c):
            # output projection for s-chunk c, split into 8 independent
            # units used as PE fillers inside later attention loops (the
            # late chunks' long j-loops have idle spill slots)
            for t in range(4):
                for n in range(2):
                    filler.append((c, t, n))

        def emit_proj_unit(c, t, n):
            ss = slice(128 * (4 * c + t), 128 * (4 * c + t + 1))
            nn = slice(512 * n, 512 * (n + 1))
            pr_ps = ps_mm.tile([128, 512], F32, tag="mm")
            nc.tensor.matmul(pr_ps, OT[:, ss], wo_sb[:, nn],
                             start=True, stop=True)
            o_sb = ost_pool.tile([128, 512], F32, tag="ost")
            nc.vector.tensor_copy(o_sb, pr_ps)
            nc.sync.dma_start(out=out[ss, nn], in_=o_sb)

        xT_k = xT.rearrange("(k p) s -> p k s", p=128)

        def load_xt(c):
            xt = xt_pool.tile([128, ND, 512], CDT, tag="xt")
            cc0 = slice(512 * c, 512 * (c + 1))
            if c == 0:
                # split the first load so the first matmul starts early
                nc.sync.dma_start(out=xt[:, 0:2], in_=xT_k[:, 0:2, cc0])
                nc.sync.dma_start(out=xt[:, 2:ND], in_=xT_k[:, 2:ND, cc0])
            else:
                nc.sync.dma_start(out=xt, in_=xT_k[:, :, cc0])
            return [xt[:, k] for k in range(ND)]

        xts = load_xt(0)
        emit_late_consts()
        for c in range(NS):
            cc = slice(512 * c, 512 * (c + 1))

            # ---- phase A: QKV projections for s-chunk c ---------------
            vt_sb = vt_pool.tile([128, 512], F32, tag="vt")
            for i, (dst, b_sb) in enumerate(((QT[:, cc], bq_sb),
                                             (KT[:, cc], bk_sb),
                                             (vt_sb, bv_sb))):
                p_ps = ps_mm.tile([128, 512], F32, tag="mm")
                for k in range(ND):
                    nc.tensor.matmul(p_ps, w_sb[:, i, k], xts[k],
                                     start=(k == 0), stop=(k == ND - 1))
                nc.vector.tensor_scalar_add(dst, p_ps, b_sb)
            # V^T [c, s] -> V natural [s, c] per 128-block, + ones column
            for t in range(4):
                j = 4 * c + t
                tr_ps = ps_mm.tile([128, 128], F32, tag="mm")
                nc.tensor.transpose(tr_ps, vt_sb[:, 128 * t:128 * (t + 1)],
                                    id_sb)
                nc.vector.tensor_copy(VP[:, 0, j, 0:64], tr_ps[:, 0:64])
                nc.vector.tensor_copy(VP[:, 1, j, 0:64], tr_ps[:, 64:128])
                # ones column (Memset has no f32r encoding: use in*0+1)
                for h in (0, 1):
                    nc.vector.tensor_scalar(VP[:, h, j, 64:65],
                                            tri_sb[:, 0, 0:1],
                                            0.0, 1.0, mybir.AluOpType.mult,
                                            mybir.AluOpType.add)

            # prefetch next chunk's activations before the long attention
            # loop, so the load isn't queued behind its filler out-DMAs
            xts_next = load_xt(c + 1) if c + 1 < NS else None

            if c > 0:
                push_proj(c - 1)

            # ---- phase B: attention for sq chunk c --------------------
            njt = 4 * (c + 1)  # causal sk tiles
            ot0 = ps_ot.tile([65, 512], F32, tag="ot")
            ot1 = ps_ot.tile([65, 512], F32, tag="ot")

            def emit_av(j, pt):
                st, sp = (j == 0), (j == njt - 1)
                nc.tensor.matmul(ot0, VP[:, 0, j], pt[:, 0:512], start=st, stop=sp)
                nc.tensor.matmul(ot1, VP[:, 1, j], pt[:, 512:1024], start=st, stop=sp)

            # software pipeline: attn@V for iteration j-1 is emitted after
            # scores+exp of iteration j, so the in-order PE queue never
            # waits on the ACT engine's exp.
            pending = []
            for j in range(njt):
                jj = slice(128 * j, 128 * (j + 1))
                sc_ps = ps_sc.tile([128, 1024], F32, tag="sc")
                nc.tensor.matmul(sc_ps[:, 0:512], KT[0:64, jj], QT[0:64, cc],
                                 start=True, stop=True)
                nc.tensor.matmul(sc_ps[:, 512:1024], KT[64:128, jj], QT[64:128, cc],
                                 start=True, stop=True)
                pt = pt_pool.tile([128, 1024], CDT, tag="pt")
                nc.scalar.activation(out=pt, in_=sc_ps, func=EXP)
                off = 128 * j - 512 * c
                if off >= 0:  # diagonal tile: mask (zeros below 128j, tri on diag)
                    t = j - 4 * c
                    for half in (0, 512):
                        nc.vector.tensor_mul(pt[:, half:half + off + 128],
                                             pt[:, half:half + off + 128],
                                             tri_sb[:, t, 0:off + 128])
                pending.append((j, pt))
                if len(pending) > 2:
                    emit_av(*pending.pop(0))
                if filler:
                    emit_proj_unit(*filler.popleft())
            for p in pending:
                emit_av(*p)

            if dbg_t and c == 1:
                for name, ot in (("d_ot0", ot0), ("d_ot1", ot1)):
                    o_cp = ost_pool.tile([65, 512], F32R, tag="dbg")
                    nc.vector.tensor_copy(o_cp, ot)
                    nc.sync.dma_start(out=dbg_t[name], in_=o_cp)

            # softmax denominators (row 64) -> rescale; stage-interleaved so
            # the DVE and gpsimd legs of the two heads pipeline
            dens, recs, bcs = [], [], []
            for h, ot in ((0, ot0), (1, ot1)):
                den = rec_pool.tile([1, 512], F32, tag="den")
                nc.vector.tensor_copy(den, ot[64:65, :])
                dens.append(den)
            for h in (0, 1):
                rec = rec_pool.tile([1, 512], F32, tag="rec")
                nc.vector._custom_dve(RECIPROCAL_APPROX_FAST, out=rec,
                                      in0=dens[h],
                                      s0=RECIP_APPROX_FAST_CONSTS["s0"],
                                      s1=RECIP_APPROX_FAST_CONSTS["s1"],
                                      imm2=RECIP_APPROX_FAST_CONSTS["imm2"])
                recs.append(rec)
                bc = bc_pool.tile([64, 512], F32, tag="bc")
                nc.gpsimd.partition_broadcast(bc, rec)
                bcs.append(bc)
            for h, ot in ((0, ot0), (1, ot1)):
                nc.vector.tensor_mul(OT[64 * h:64 * (h + 1), cc], ot[0:64, :],
                                     bcs[h])
            xts = xts_next

        while filler:
            emit_proj_unit(*filler.popleft())
        push_proj(NS - 1)
        while filler:
            emit_proj_unit(*filler.popleft())

        if dbg_t:
            nc.sync.dma_start(out=dbg_t["d_QT"], in_=QT)
            nc.sync.dma_start(out=dbg_t["d_KT"], in_=KT)
            nc.sync.dma_start(out=dbg_t["d_VP"], in_=VP)
            nc.sync.dma_start(out=dbg_t["d_OT0"], in_=OT[0:64])
            nc.sync.dma_start(out=dbg_t["d_OT1"], in_=OT[64:128])


_NC_CACHE = {}


def _get_nc():
    if "nc" not in _NC_CACHE:
        _NC_CACHE["nc"] = _build_nc()
    return _NC_CACHE["nc"]


def make_in_maps(x, Wq, bq, Wk, bk, Wv, bv, Wo, bo):
    import ml_dtypes
    cdt = ml_dtypes.bfloat16 if USE_BF16 else np.float32
    x = np.asarray(x, np.float32).reshape(S, D)
    xT = np.ascontiguousarray(x.T).astype(cdt)
    scale = 1.0 / np.sqrt(HD)
    # tri[:, t, :]: zeros on cols < 128t, upper-tri on cols [128t, 128t+128),
    # ones beyond (multiplied region is cols [0, 128t+128) of a 512 chunk)
    tri = np.ones((128, 4, 512), np.float32)
    triu = np.triu(np.ones((128, 128), np.float32))
    for t in range(4):
        tri[:, t, :128 * t] = 0.0
        tri[:, t, 128 * t:128 * (t + 1)] = triu
    ident = np.eye(128, dtype=np.float32)
    in_maps = []
    for c in range(NCORES):
        cs = slice(CPC * c, CPC * (c + 1))
        # w3[p, proj, k, c2] = W[128k+p, c2] for the three projections
        w3 = np.stack([np.asarray(Wq)[:, cs] * scale,
                       np.asarray(Wk)[:, cs],
                       np.asarray(Wv)[:, cs]], axis=1)  # [D, 3, 128]
        w3 = np.ascontiguousarray(
            w3.reshape(ND, 128, 3, CPC).transpose(1, 2, 0, 3)).astype(cdt)
        in_maps.append({
            "xT": xT,
            "w3": w3,
            "wo": np.ascontiguousarray(np.asarray(Wo)[cs, :]).astype(cdt),
            "bq": np.ascontiguousarray(np.asarray(bq)[cs] * scale).reshape(CPC, 1),
            "bk": np.ascontiguousarray(np.asarray(bk)[cs]).reshape(CPC, 1),
            "bv": np.ascontiguousarray(np.asarray(bv)[cs]).reshape(CPC, 1),
            "tri": tri.astype(cdt),
            "ident": ident,
        })
    return in_maps


def kernel(x, Wq, bq, Wk, bk, Wv, bv, Wo, bo, _run_kwargs=None):
    nc = _get_nc()
    in_maps = make_in_maps(x, Wq, bq, Wk, bk, Wv, bv, Wo, bo)
    res = run_bass_kernel_spmd(nc, in_maps, list(range(NCORES)),
                               **(_run_kwargs or {}))
    acc = np.zeros((S, D), np.float64)
    for c in range(NCORES):
        acc += res.results[c]["out"]
    full = (acc + np.asarray(bo, np.float64)).astype(np.float32)
    if _run_kwargs is not None:
        _NC_CACHE["last_results"] = res
    return full.reshape(1, S, D)



# revision 11
# speedup vs baseline: 1.0271x; 1.0271x over previous
"""Causal multi-head attention (B=1, S=4096, D=1024, H=16, HD=64) on 8 TRN2
NeuronCores.

Sharding: tensor-parallel over heads. Core c owns heads [2c, 2c+1]: Wq/Wk/Wv
column slices (128 cols) and Wo row slice (128 rows). Each core computes its
partial output projection over the full sequence in bf16; the host sums the 8
partials and adds bo (the row-parallel all-reduce done at gather time).

v3 design notes (evolution of the v1 tensor-parallel kernel):
  - The attention j-loop is ACT(exp)-bound: exp of [128, 1024] costs
    (N+352)/1.2 ns ~= 1.15us per 128-row sk tile, and 144 tiles/core is
    ~155us of ACT time. Everything else is packed underneath it.
  - attn@V is mixed-precision: OFF-DIAGONAL sk tiles (the bulk) run as
    fp8 DoubleRow matmuls over PAIRS of sk tiles (K=256/instr, V in
    e4m3, softmax weights in e5m2 straight from ACT exp), halving PE
    time; the 4 DIAGONAL tiles of each chunk run per-j in bf16, because
    the dominant self-attention weights live there and fp8 noise on
    them doesn't average out (off-diagonal weights are small relative
    to the row total, so their fp8 noise is harmless). The softmax
    denominator rides along as a ones column (row 64 of the
    accumulator) in both paths.
  - causal diagonal tiles are shrunk: scores matmuls, exp, and attn@V
    only cover valid columns; masking is a pre-exp additive -1e9 on the
    128x128 diagonal band (DVE on PSUM), so no post-exp multiply.
  - QKV projections + V transposes for chunk c+1 are emitted as filler
    units inside chunk c's attention loop (interleaved with the
    deferred output-projection units), so the PE absorbs them in the
    slack under exp and ACT never waits at chunk boundaries. x^T
    activations are prefetched two chunks ahead.
  - V^T -> V via bf16 PE transposes; output partials written bf16.
"""
import sys

sys.path.insert(0, "/opt/trn_rl_repo")

import numpy as np

import concourse.bacc as bacc
import concourse.mybir as mybir
import concourse.tile as tile
from concourse.bass_utils import run_bass_kernel_spmd
from concourse.dve_ops import RECIPROCAL_APPROX_FAST, RECIP_APPROX_FAST_CONSTS

F32 = mybir.dt.float32
BF16 = mybir.dt.bfloat16
FP8 = mybir.dt.float8e4
FP8E5 = mybir.dt.float8e5
CDT = BF16                      # bf16 compute dtype for projections/scores
EXP = mybir.ActivationFunctionType.Exp
DR = mybir.MatmulPerfMode.DoubleRow
USE_FP8_AV = True               # fp8 DoubleRow for off-diagonal attn@V pairs

S, D, H, HD = 4096, 1024, 16, 64
NCORES = 8
CPC = D // NCORES  # 128 head-dim columns per core (2 heads)
NS = S // 512      # 8 chunks of 512 along the sequence
ND = D // 128      # 8 contraction chunks for the projections
NJ = S // 128      # 32 sk tiles


def _build_nc():
    nc = bacc.Bacc("TRN2", target_bir_lowering=False, debug=False,
                   num_devices=NCORES)
    xT = nc.dram_tensor("xT", [D, S], CDT, kind="ExternalInput").ap()
    w3 = nc.dram_tensor("w3", [128, 3, ND, 128], CDT, kind="ExternalInput").ap()
    wo = nc.dram_tensor("wo", [128, D], CDT, kind="ExternalInput").ap()
    bq = nc.dram_tensor("bq", [CPC, 1], F32, kind="ExternalInput").ap()
    bk = nc.dram_tensor("bk", [CPC, 1], F32, kind="ExternalInput").ap()
    bv = nc.dram_tensor("bv", [CPC, 1], F32, kind="ExternalInput").ap()
    tri2 = nc.dram_tensor("tri2", [128, 2, 128], CDT, kind="ExternalInput").ap()
    ident = nc.dram_tensor("ident", [128, 128], CDT, kind="ExternalInput").ap()
    out = nc.dram_tensor("out", [S, D], F32, kind="ExternalOutput").ap()

    with tile.TileContext(nc) as tc:
        _emit(nc, tc, xT, w3, wo, bq, bk, bv, tri2, ident, out)
    nc.compile()
    return nc


def _emit(nc, tc, xT, w3, wo, bq, bk, bv, tri2, ident, out):
    from collections import deque
    from contextlib import ExitStack
    ctx = ExitStack()
    with ctx:
        consts = ctx.enter_context(tc.tile_pool(name="consts", bufs=1))
        persist = ctx.enter_context(tc.tile_pool(name="persist", bufs=1))
        xt_pool = ctx.enter_context(tc.tile_pool(name="xt", bufs=4))
        vt_pool = ctx.enter_context(tc.tile_pool(name="vt", bufs=2))
        pt_pool = ctx.enter_context(tc.tile_pool(name="pt", bufs=4))
        rec_pool = ctx.enter_context(tc.tile_pool(name="rec", bufs=4))
        bc_pool = ctx.enter_context(tc.tile_pool(name="bc", bufs=4))
        ost_pool = ctx.enter_context(tc.tile_pool(name="ost", bufs=8))
        # PSUM budget (16KB/partition): mm 2x2KB + sc 2x4KB + ot 2x2KB = 16KB
        ps_mm = ctx.enter_context(tc.tile_pool(name="psmm", bufs=2, space="PSUM"))
        ps_sc = ctx.enter_context(tc.tile_pool(name="pssc", bufs=2, space="PSUM"))
        ps_ot = ctx.enter_context(tc.tile_pool(name="psot", bufs=2, space="PSUM"))

        # ---- constants (wq slice first so the first matmul starts early) ---
        w_sb = consts.tile([128, 3, ND, 128], CDT)
        nc.sync.dma_start(out=w_sb[:, 0], in_=w3[:, 0])
        bq_sb = consts.tile([CPC, 1], F32)
        bk_sb = consts.tile([CPC, 1], F32)
        bv_sb = consts.tile([CPC, 1], F32)
        nc.sync.dma_start(out=bq_sb, in_=bq)
        nc.sync.dma_start(out=bk_sb, in_=bk)
        nc.sync.dma_start(out=bv_sb, in_=bv)
        id_sb = consts.tile([128, 128], CDT)
        nb_sb = consts.tile([128, 1], F32)   # exp bias: shift scores by -3 so
        nc.vector.memset(nb_sb, -3.0)        # exp fits fp8e4m3 (cancels in the
                                             # softmax ratio via the ones-col den)
        tri_sb = consts.tile([128, 2, 128], CDT)
        wo_sb = consts.tile([128, D], CDT)

        def emit_late_consts():
            nc.sync.dma_start(out=w_sb[:, 1], in_=w3[:, 1])
            nc.sync.dma_start(out=w_sb[:, 2], in_=w3[:, 2])
            nc.sync.dma_start(out=tri_sb, in_=tri2)
            nc.sync.dma_start(out=id_sb, in_=ident)
            nc.sync.dma_start(out=wo_sb, in_=wo)

        # ---- persistent activations -----------------------------------
        QT = persist.tile([128, S], CDT)      # [c(2 heads x 64), s]
        KT = persist.tile([128, S], CDT)
        # V natural + ones col, bf16 (diagonal per-j use): [p, head, j, 65]
        VPH = persist.tile([128, 2, NJ, 65], CDT)
        # fp8 copy for off-diagonal DoubleRow pairs: [p, member, jp, head, 65]
        VP8 = persist.tile([128, 2, NJ // 2, 2, 65], FP8)
        OT = persist.tile([128, S], CDT)      # rescaled attn out^T, both heads

        filler = deque()  # (kind, fn) units drained 1-per-j inside the loop

        xT_k = xT.rearrange("(k p) s -> p k s", p=128)
        xts = {}

        def load_xt(c):
            xt = xt_pool.tile([128, ND, 512], CDT, tag="xt", name="xt")
            cc0 = slice(512 * c, 512 * (c + 1))
            if c == 0:
                # split the first load so the first matmul starts early
                nc.sync.dma_start(out=xt[:, 0:2], in_=xT_k[:, 0:2, cc0])
                nc.sync.dma_start(out=xt[:, 2:ND], in_=xT_k[:, 2:ND, cc0])
            else:
                nc.sync.dma_start(out=xt, in_=xT_k[:, :, cc0])
            return [xt[:, k] for k in range(ND)]

        def make_qkv_units(c):
            # QKV projections + V transposes for chunk c, split into ~0.4-0.9us
            # PE units usable as fillers. Each psum-allocating unit emits its
            # consumer before two more "mm" allocations occur (ring safety).
            cc = slice(512 * c, 512 * (c + 1))
            st = {}
            units = []

            def mk_half(i, kh):
                def f():
                    if kh == 0:
                        st[i] = ps_mm.tile([128, 512], F32, tag="mm", name="qkvps")
                    p_ps = st[i]
                    for k in range(4 * kh, 4 * kh + 4):
                        nc.tensor.matmul(p_ps, w_sb[:, i, k], xts[c][k],
                                         start=(k == 0), stop=(k == ND - 1))
                    if kh == 1:
                        if i == 0:
                            dst, b_sb = QT[:, cc], bq_sb
                        elif i == 1:
                            dst, b_sb = KT[:, cc], bk_sb
                        else:
                            st["vt"] = vt_pool.tile([128, 512], CDT, tag="vt",
                                                    name="vt")
                            dst, b_sb = st["vt"], bv_sb
                        nc.vector.tensor_scalar_add(dst, p_ps, b_sb)
                return f

            for i in range(3):
                units.append(mk_half(i, 0))
                units.append(mk_half(i, 1))

            def mk_tr(t):
                def f():
                    j = 4 * c + t
                    tr = ps_mm.tile([128, 128], CDT, tag="mm", name="trps")
                    nc.tensor.transpose(tr, st["vt"][:, 128 * t:128 * (t + 1)],
                                        id_sb)
                    tr3 = tr.rearrange("p (h d) -> p h d", h=2)
                    nc.vector.tensor_copy(VPH[:, :, j, 0:64], tr3)
                    # ones column (exact in bf16/fp8): in*0 + 1
                    nc.vector.tensor_scalar(VPH[:, :, j, 64:65],
                                            tr3[:, :, 0:1], 0.0, 1.0,
                                            mybir.AluOpType.mult,
                                            mybir.AluOpType.add)
                    if USE_FP8_AV:
                        nc.vector.tensor_copy(VP8[:, j % 2, j // 2, :, 0:64],
                                              tr3)
                        nc.vector.tensor_scalar(VP8[:, j % 2, j // 2, :, 64:65],
                                                tr3[:, :, 0:1], 0.0, 1.0,
                                                mybir.AluOpType.mult,
                                                mybir.AluOpType.add)
                return f

            for t in range(4):
                units.append(mk_tr(t))
            return units

        def push_proj(c):
            # output projection for s-chunk c, 8 filler units
            for t in range(4):
                for n in range(2):
                    def f(c=c, t=t, n=n):
                        ss = slice(128 * (4 * c + t), 128 * (4 * c + t + 1))
                        nn = slice(512 * n, 512 * (n + 1))
                        pr_ps = ps_mm.tile([128, 512], F32, tag="mm", name="prps")
                        nc.tensor.matmul(pr_ps, OT[:, ss], wo_sb[:, nn],
                                         start=True, stop=True)
                        o_sb = ost_pool.tile([128, 512], F32, tag="ost",
                                             name="osb")
                        nc.vector.tensor_copy(o_sb, pr_ps)
                        nc.sync.dma_start(out=out[ss, nn], in_=o_sb)
                    filler.append(("proj", f))

        def pop_filler():
            if filler:
                filler.popleft()[1]()

        # ---- bootstrap ------------------------------------------------
        xts[0] = load_xt(0)
        emit_late_consts()
        xts[1] = load_xt(1)
        for u in make_qkv_units(0):
            u()

        for c in range(NS):
            cc = slice(512 * c, 512 * (c + 1))
            if c + 2 < NS:
                xts[c + 2] = load_xt(c + 2)
            if c + 1 < NS:
                filler.extend(("qkv", u) for u in make_qkv_units(c + 1))

            njt = 4 * (c + 1)
            ot0 = ps_ot.tile([128, 512], F32, tag="ot", name="ot0")
            ot1 = ps_ot.tile([128, 512], F32, tag="ot", name="ot1")

            def emit_av_pair(jp, ptp, _off, ot0=ot0, ot1=ot1):
                # off-diagonal pair: full width, K=256 fp8 DoubleRow
                st_ = (jp == 0)
                p4 = ptp.rearrange("p m (h q) -> p m h q", h=2)
                nc.tensor.matmul(ot0[0:65, :], VP8[:, :, jp, 0, :],
                                 p4[:, :, 0, :], start=st_, stop=False,
                                 perf_mode=DR)
                nc.tensor.matmul(ot1[0:65, :], VP8[:, :, jp, 1, :],
                                 p4[:, :, 1, :], start=st_, stop=False,
                                 perf_mode=DR)

            def emit_av_diag(j, ptd, off, c=c, njt=njt, ot0=ot0, ot1=ot1):
                st_ = (j == 0)          # only chunk 0 starts on a diagonal
                sp = (j == njt - 1)
                pt3 = ptd.rearrange("p (h q) -> p h q", h=2)
                nc.tensor.matmul(ot0[0:65, off:512], VPH[:, 0, j],
                                 pt3[:, 0, off:512], start=st_, stop=sp)
                nc.tensor.matmul(ot1[0:65, off:512], VPH[:, 1, j],
                                 pt3[:, 1, off:512], start=st_, stop=sp)

            def emit_av_single(j, pt2, off, njt=njt, ot0=ot0, ot1=ot1):
                # bf16 fallback path (USE_FP8_AV=False): per-j for all tiles
                st_, sp = (j == 0), (j == njt - 1)
                pt3 = pt2.rearrange("p (h q) -> p h q", h=2)
                nc.tensor.matmul(ot0[0:65, off:512], VPH[:, 0, j],
                                 pt3[:, 0, off:512], start=st_, stop=sp)
                nc.tensor.matmul(ot1[0:65, off:512], VPH[:, 1, j],
                                 pt3[:, 1, off:512], start=st_, stop=sp)

            pending = []
            ptp_cur = None
            for j in range(njt):
                jj = slice(128 * j, 128 * (j + 1))
                t = j - 4 * c
                off = 128 * t if t >= 0 else 0
                sc = ps_sc.tile([128, 1024], F32, tag="sc", name="sc")
                sc3 = sc.rearrange("p (h q) -> p h q", h=2)
                nc.tensor.matmul(sc3[:, 0, off:512], KT[0:64, jj],
                                 QT[0:64, 512 * c + off:512 * (c + 1)],
                                 start=True, stop=True)
                nc.tensor.matmul(sc3[:, 1, off:512], KT[64:128, jj],
                                 QT[64:128, 512 * c + off:512 * (c + 1)],
                                 start=True, stop=True)
                if USE_FP8_AV and t < 0:
                    m = j % 2
                    if m == 0:
                        ptp_cur = pt_pool.tile([128, 2, 1024], FP8,
                                               tag="pt8", name="ptp")
                    pt3 = ptp_cur[:, m].rearrange("p (h q) -> p h q", h=2)
                    nc.scalar.activation(out=pt3[:, :, :], in_=sc3[:, :, :],
                                         func=EXP, bias=nb_sb)
                    if m == 1:
                        pending.append(("pair", j // 2, ptp_cur, 0))
                else:
                    ptd = pt_pool.tile([128, 1024], CDT, tag="ptd", name="ptd")
                    pt3 = ptd.rearrange("p (h q) -> p h q", h=2)
                    nc.scalar.activation(out=pt3[:, :, off:512],
                                         in_=sc3[:, :, off:512], func=EXP,
                                         bias=nb_sb)
                    if t >= 0:
                        # diagonal band: post-exp multiplicative triu 0/1 mask
                        # on bf16 SBUF (in-place, same pattern as v1)
                        nc.vector.tensor_mul(pt3[:, :, off:off + 128],
                                             pt3[:, :, off:off + 128], tri_sb)
                    kind = "diag" if USE_FP8_AV else "single"
                    pending.append((kind, j, ptd, off))
                while len(pending) > 2:
                    kind, a, b, o2 = pending.pop(0)
                    if kind == "pair":
                        emit_av_pair(a, b, o2)
                    elif kind == "diag":
                        emit_av_diag(a, b, o2)
                    else:
                        emit_av_single(a, b, o2)
                pop_filler()
            for kind, a, b, o2 in pending:
                if kind == "pair":
                    emit_av_pair(a, b, o2)
                elif kind == "diag":
                    emit_av_diag(a, b, o2)
                else:
                    emit_av_single(a, b, o2)

            # any undrained QKV units for c+1 must run before its attention
            while filler and filler[0][0] == "qkv":
                filler.popleft()[1]()

            # softmax denominators (row 64) -> rescale OT
            recs, bcs = [], []
            for ot in (ot0, ot1):
                rec = rec_pool.tile([1, 512], F32, tag="rec", name="rec")
                nc.vector._custom_dve(RECIPROCAL_APPROX_FAST, out=rec,
                                      in0=ot[64:65, :],
                                      s0=RECIP_APPROX_FAST_CONSTS["s0"],
                                      s1=RECIP_APPROX_FAST_CONSTS["s1"],
                                      imm2=RECIP_APPROX_FAST_CONSTS["imm2"])
                recs.append(rec)
            for h in (0, 1):
                bc = bc_pool.tile([64, 512], F32, tag="bc", name="bc")
                nc.gpsimd.partition_broadcast(bc, recs[h])
                bcs.append(bc)
            for h, ot in ((0, ot0), (1, ot1)):
                nc.vector.tensor_mul(OT[64 * h:64 * (h + 1), cc], ot[0:64, :],
                                     bcs[h])
            push_proj(c)

        while filler:
            filler.popleft()[1]()


_NC_CACHE = {}


def _get_nc():
    if "nc" not in _NC_CACHE:
        _NC_CACHE["nc"] = _build_nc()
    return _NC_CACHE["nc"]


def make_in_maps(x, Wq, bq, Wk, bk, Wv, bv, Wo, bo):
    import ml_dtypes
    cdt = ml_dtypes.bfloat16
    x = np.asarray(x, np.float32).reshape(S, D)
    xT = np.ascontiguousarray(x.T).astype(cdt)
    scale = 1.0 / np.sqrt(HD)
    # additive causal mask for the 128x128 diagonal band (transposed scores:
    # pt[sk, q] masked iff sk > q within the band)
    r = np.arange(128)
    tri2 = np.where(r[:, None] > r[None, :], 0.0, 1.0).astype(np.float32)
    tri2 = np.ascontiguousarray(
        np.broadcast_to(tri2[:, None, :], (128, 2, 128))).astype(cdt)
    ident = np.eye(128, dtype=np.float32).astype(cdt)
    in_maps = []
    for c in range(NCORES):
        cs = slice(CPC * c, CPC * (c + 1))
        # w3[p, proj, k, c2] = W[128k+p, c2] for the three projections
        w3 = np.stack([np.asarray(Wq)[:, cs] * scale,
                       np.asarray(Wk)[:, cs],
                       np.asarray(Wv)[:, cs]], axis=1)  # [D, 3, 128]
        w3 = np.ascontiguousarray(
            w3.reshape(ND, 128, 3, CPC).transpose(1, 2, 0, 3)).astype(cdt)
        in_maps.append({
            "xT": xT,
            "w3": w3,
            "wo": np.ascontiguousarray(np.asarray(Wo)[cs, :]).astype(cdt),
            "bq": np.ascontiguousarray(np.asarray(bq)[cs] * scale).reshape(CPC, 1),
            "bk": np.ascontiguousarray(np.asarray(bk)[cs]).reshape(CPC, 1),
            "bv": np.ascontiguousarray(np.asarray(bv)[cs]).reshape(CPC, 1),
            "tri2": tri2,
            "ident": ident,
        })
    return in_maps


def kernel(x, Wq, bq, Wk, bk, Wv, bv, Wo, bo, _run_kwargs=None):
    nc = _get_nc()
    in_maps = make_in_maps(x, Wq, bq, Wk, bk, Wv, bv, Wo, bo)
    res = run_bass_kernel_spmd(nc, in_maps, list(range(NCORES)),
                               **(_run_kwargs or {}))
    acc = np.zeros((S, D), np.float64)
    for c in range(NCORES):
        acc += np.asarray(res.results[c]["out"], np.float64)
    full = (acc + np.asarray(bo, np.float64)).astype(np.float32)
    if _run_kwargs is not None:
        _NC_CACHE["last_results"] = res
    return full.reshape(1, S, D)


# revision 13
# speedup vs baseline: 1.1363x; 1.1063x over previous
"""Causal multi-head attention (B=1, S=4096, D=1024, H=16, HD=64) on 8 TRN2
NeuronCores.

Sharding: tensor-parallel over heads. Core c owns heads [2c, 2c+1]: Wq/Wk/Wv
column slices (128 cols) and Wo row slice (128 rows). Each core computes its
partial output projection over the full sequence in bf16; the host sums the 8
partials and adds bo (the row-parallel all-reduce done at gather time).

v3 design notes (evolution of the v1 tensor-parallel kernel):
  - The attention j-loop is ACT(exp)-bound: exp of [128, 1024] costs
    (N+352)/1.2 ns ~= 1.15us per 128-row sk tile, and 144 tiles/core is
    ~155us of ACT time. Everything else is packed underneath it.
  - attn@V is mixed-precision: OFF-DIAGONAL sk tiles (the bulk) run as
    fp8 DoubleRow matmuls over PAIRS of sk tiles (K=256/instr, V in
    e4m3, softmax weights in e5m2 straight from ACT exp), halving PE
    time; the 4 DIAGONAL tiles of each chunk run per-j in bf16, because
    the dominant self-attention weights live there and fp8 noise on
    them doesn't average out (off-diagonal weights are small relative
    to the row total, so their fp8 noise is harmless). The softmax
    denominator rides along as a ones column (row 64 of the
    accumulator) in both paths.
  - causal diagonal tiles are shrunk: scores matmuls, exp, and attn@V
    only cover valid columns; masking is a post-exp multiplicative triu
    0/1 on just the 128x128 diagonal band of the bf16 weights. Scores
    are shifted by -3 before exp (exp bias) so the off-diagonal weights
    fit fp8e4m3 range; the shift cancels in the softmax ratio.
  - QKV projections + V transposes for chunk c+1 are emitted as filler
    units inside chunk c's attention loop (interleaved with the
    deferred output-projection units), so the PE absorbs them in the
    slack under exp and ACT never waits at chunk boundaries. x^T
    activations are prefetched two chunks ahead.
  - V^T -> V via bf16 PE transposes; output partials written bf16.
"""
import sys

sys.path.insert(0, "/opt/trn_rl_repo")

import numpy as np

import concourse.bacc as bacc
import concourse.mybir as mybir
import concourse.tile as tile
from concourse.bass_utils import run_bass_kernel_spmd
from concourse.dve_ops import RECIPROCAL_APPROX_FAST, RECIP_APPROX_FAST_CONSTS

F32 = mybir.dt.float32
BF16 = mybir.dt.bfloat16
FP8 = mybir.dt.float8e4
FP8E5 = mybir.dt.float8e5
CDT = BF16                      # bf16 compute dtype for projections/scores
EXP = mybir.ActivationFunctionType.Exp
DR = mybir.MatmulPerfMode.DoubleRow
USE_FP8_AV = True               # fp8 DoubleRow for off-diagonal attn@V pairs

S, D, H, HD = 4096, 1024, 16, 64
NCORES = 8
CPC = D // NCORES  # 128 head-dim columns per core (2 heads)
NS = S // 512      # 8 chunks of 512 along the sequence
ND = D // 128      # 8 contraction chunks for the projections
NJ = S // 128      # 32 sk tiles


def _build_nc():
    nc = bacc.Bacc("TRN2", target_bir_lowering=False, debug=False,
                   num_devices=NCORES)
    xT = nc.dram_tensor("xT", [D, S], CDT, kind="ExternalInput").ap()
    w3 = nc.dram_tensor("w3", [128, 3, ND, 128], CDT, kind="ExternalInput").ap()
    wo = nc.dram_tensor("wo", [128, D], CDT, kind="ExternalInput").ap()
    bq = nc.dram_tensor("bq", [CPC, 1], F32, kind="ExternalInput").ap()
    bk = nc.dram_tensor("bk", [CPC, 1], F32, kind="ExternalInput").ap()
    bv = nc.dram_tensor("bv", [CPC, 1], F32, kind="ExternalInput").ap()
    tri2 = nc.dram_tensor("tri2", [128, 2, 128], CDT, kind="ExternalInput").ap()
    ident = nc.dram_tensor("ident", [128, 128], CDT, kind="ExternalInput").ap()
    out = nc.dram_tensor("out", [S, D], F32, kind="ExternalOutput").ap()

    with tile.TileContext(nc) as tc:
        _emit(nc, tc, xT, w3, wo, bq, bk, bv, tri2, ident, out)
    nc.compile()
    return nc


def _emit(nc, tc, xT, w3, wo, bq, bk, bv, tri2, ident, out):
    from collections import deque
    from contextlib import ExitStack
    ctx = ExitStack()
    with ctx:
        consts = ctx.enter_context(tc.tile_pool(name="consts", bufs=1))
        persist = ctx.enter_context(tc.tile_pool(name="persist", bufs=1))
        xt_pool = ctx.enter_context(tc.tile_pool(name="xt", bufs=4))
        vt_pool = ctx.enter_context(tc.tile_pool(name="vt", bufs=2))
        pt_pool = ctx.enter_context(tc.tile_pool(name="pt", bufs=6))
        rec_pool = ctx.enter_context(tc.tile_pool(name="rec", bufs=4))
        bc_pool = ctx.enter_context(tc.tile_pool(name="bc", bufs=4))
        ost_pool = ctx.enter_context(tc.tile_pool(name="ost", bufs=8))
        # PSUM budget (16KB/partition): mm 2x2KB + sc 2x4KB + ot 2x2KB = 16KB
        ps_mm = ctx.enter_context(tc.tile_pool(name="psmm", bufs=2, space="PSUM"))
        ps_sc = ctx.enter_context(tc.tile_pool(name="pssc", bufs=2, space="PSUM"))
        ps_ot = ctx.enter_context(tc.tile_pool(name="psot", bufs=2, space="PSUM"))

        # ---- constants (wq slice first so the first matmul starts early) ---
        w_sb = consts.tile([128, 3, ND, 128], CDT)
        nc.sync.dma_start(out=w_sb[:, 0], in_=w3[:, 0])
        bq_sb = consts.tile([CPC, 1], F32)
        bk_sb = consts.tile([CPC, 1], F32)
        bv_sb = consts.tile([CPC, 1], F32)
        nc.sync.dma_start(out=bq_sb, in_=bq)
        nc.sync.dma_start(out=bk_sb, in_=bk)
        nc.sync.dma_start(out=bv_sb, in_=bv)
        id_sb = consts.tile([128, 128], CDT)
        nb_sb = consts.tile([128, 1], F32)   # exp bias: shift scores by -3 so
        nc.vector.memset(nb_sb, -3.0)        # exp fits fp8e4m3 (cancels in the
                                             # softmax ratio via the ones-col den)
        tri_sb = consts.tile([128, 2, 128], CDT)
        wo_sb = consts.tile([128, D], CDT)

        def emit_late_consts():
            nc.sync.dma_start(out=w_sb[:, 1], in_=w3[:, 1])
            nc.sync.dma_start(out=w_sb[:, 2], in_=w3[:, 2])
            nc.sync.dma_start(out=tri_sb, in_=tri2)
            nc.sync.dma_start(out=id_sb, in_=ident)
            nc.sync.dma_start(out=wo_sb, in_=wo)

        # ---- persistent activations -----------------------------------
        QT = persist.tile([128, S], CDT)      # [c(2 heads x 64), s]
        KT = persist.tile([128, S], CDT)
        # V natural + ones col, bf16 (diagonal per-j use): [p, head, j, 65]
        VPH = persist.tile([128, 2, NJ, 65], CDT)
        # fp8 copy for off-diagonal DoubleRow pairs: [p, member, jp, head, 65]
        VP8 = persist.tile([128, 2, NJ // 2, 2, 65], FP8)
        OT = persist.tile([128, S], CDT)      # rescaled attn out^T, both heads

        filler_q = deque()  # QKV units for chunk c+1 (hard deadline: popped first)
        filler_p = deque()  # deferred output-projection units (no deadline)

        xT_k = xT.rearrange("(k p) s -> p k s", p=128)
        xts = {}

        def load_xt(c):
            xt = xt_pool.tile([128, ND, 512], CDT, tag="xt", name="xt")
            cc0 = slice(512 * c, 512 * (c + 1))
            if c == 0:
                # split the first load so the first matmul starts early
                nc.sync.dma_start(out=xt[:, 0:2], in_=xT_k[:, 0:2, cc0])
                nc.sync.dma_start(out=xt[:, 2:ND], in_=xT_k[:, 2:ND, cc0])
            else:
                nc.sync.dma_start(out=xt, in_=xT_k[:, :, cc0])
            return [xt[:, k] for k in range(ND)]

        def make_qkv_units(c):
            # QKV projections + V transposes for chunk c, split into ~0.4-0.9us
            # PE units usable as fillers. Each psum-allocating unit emits its
            # consumer before two more "mm" allocations occur (ring safety).
            cc = slice(512 * c, 512 * (c + 1))
            st = {}
            units = []

            def mk_quarter(i, kq):
                def f():
                    if kq == 0:
                        st[i] = ps_mm.tile([128, 512], F32, tag="mm", name="qkvps")
                    p_ps = st[i]
                    for k in range(2 * kq, 2 * kq + 2):
                        nc.tensor.matmul(p_ps, w_sb[:, i, k], xts[c][k],
                                         start=(k == 0), stop=(k == ND - 1))
                    if kq == 3:
                        if i == 0:
                            dst, b_sb = QT[:, cc], bq_sb
                        elif i == 1:
                            dst, b_sb = KT[:, cc], bk_sb
                        else:
                            st["vt"] = vt_pool.tile([128, 512], CDT, tag="vt",
                                                    name="vt")
                            dst, b_sb = st["vt"], bv_sb
                        nc.vector.tensor_scalar_add(dst, p_ps, b_sb)
                return f

            for i in range(3):
                for kq in range(4):
                    units.append(mk_quarter(i, kq))

            def mk_tr(t):
                def f():
                    j = 4 * c + t
                    tr = ps_mm.tile([128, 128], CDT, tag="mm", name="trps")
                    nc.tensor.transpose(tr, st["vt"][:, 128 * t:128 * (t + 1)],
                                        id_sb)
                    tr3 = tr.rearrange("p (h d) -> p h d", h=2)
                    nc.vector.tensor_copy(VPH[:, :, j, 0:64], tr3)
                    # ones column (exact in bf16/fp8): in*0 + 1
                    nc.vector.tensor_scalar(VPH[:, :, j, 64:65],
                                            tr3[:, :, 0:1], 0.0, 1.0,
                                            mybir.AluOpType.mult,
                                            mybir.AluOpType.add)
                    if USE_FP8_AV:
                        nc.vector.tensor_copy(VP8[:, j % 2, j // 2, :, 0:64],
                                              tr3)
                        nc.vector.tensor_scalar(VP8[:, j % 2, j // 2, :, 64:65],
                                                tr3[:, :, 0:1], 0.0, 1.0,
                                                mybir.AluOpType.mult,
                                                mybir.AluOpType.add)
                return f

            for t in range(4):
                units.append(mk_tr(t))
            return units

        def push_proj(c):
            # output projection for s-chunk c, 8 filler units
            for t in range(4):
                for n in range(2):
                    def f(c=c, t=t, n=n):
                        ss = slice(128 * (4 * c + t), 128 * (4 * c + t + 1))
                        nn = slice(512 * n, 512 * (n + 1))
                        pr_ps = ps_mm.tile([128, 512], F32, tag="mm", name="prps")
                        nc.tensor.matmul(pr_ps, OT[:, ss], wo_sb[:, nn],
                                         start=True, stop=True)
                        o_sb = ost_pool.tile([128, 512], F32, tag="ost",
                                             name="osb")
                        nc.vector.tensor_copy(o_sb, pr_ps)
                        nc.sync.dma_start(out=out[ss, nn], in_=o_sb)
                    filler_p.append(f)

        def pop_filler(n=1):
            for _ in range(n):
                if filler_q:
                    filler_q.popleft()()
                elif filler_p:
                    filler_p.popleft()()

        # ---- bootstrap ------------------------------------------------
        xts[0] = load_xt(0)
        emit_late_consts()
        xts[1] = load_xt(1)
        for u in make_qkv_units(0):
            u()

        for c in range(NS):
            cc = slice(512 * c, 512 * (c + 1))
            if c + 2 < NS:
                xts[c + 2] = load_xt(c + 2)
            if c + 1 < NS:
                filler_q.extend(make_qkv_units(c + 1))

            njt = 4 * (c + 1)
            ot0 = ps_ot.tile([128, 512], F32, tag="ot", name="ot0")
            ot1 = ps_ot.tile([128, 512], F32, tag="ot", name="ot1")

            def emit_av_pair(jp, ptp, _off, ot0=ot0, ot1=ot1):
                # off-diagonal pair: full width, K=256 fp8 DoubleRow
                st_ = (jp == 0)
                p4 = ptp.rearrange("p m (h q) -> p m h q", h=2)
                nc.tensor.matmul(ot0[0:65, :], VP8[:, :, jp, 0, :],
                                 p4[:, :, 0, :], start=st_, stop=False,
                                 perf_mode=DR)
                nc.tensor.matmul(ot1[0:65, :], VP8[:, :, jp, 1, :],
                                 p4[:, :, 1, :], start=st_, stop=False,
                                 perf_mode=DR)

            def emit_av_diag(j, ptd, off, c=c, njt=njt, ot0=ot0, ot1=ot1):
                st_ = (j == 0)          # only chunk 0 starts on a diagonal
                sp = (j == njt - 1)
                pt3 = ptd.rearrange("p (h q) -> p h q", h=2)
                nc.tensor.matmul(ot0[0:65, off:512], VPH[:, 0, j],
                                 pt3[:, 0, off:512], start=st_, stop=sp)
                nc.tensor.matmul(ot1[0:65, off:512], VPH[:, 1, j],
                                 pt3[:, 1, off:512], start=st_, stop=sp)

            def emit_av_single(j, pt2, off, njt=njt, ot0=ot0, ot1=ot1):
                # bf16 fallback path (USE_FP8_AV=False): per-j for all tiles
                st_, sp = (j == 0), (j == njt - 1)
                pt3 = pt2.rearrange("p (h q) -> p h q", h=2)
                nc.tensor.matmul(ot0[0:65, off:512], VPH[:, 0, j],
                                 pt3[:, 0, off:512], start=st_, stop=sp)
                nc.tensor.matmul(ot1[0:65, off:512], VPH[:, 1, j],
                                 pt3[:, 1, off:512], start=st_, stop=sp)

            pending = []
            ptp_cur = None
            for j in range(njt):
                jj = slice(128 * j, 128 * (j + 1))
                t = j - 4 * c
                off = 128 * t if t >= 0 else 0
                sc = ps_sc.tile([128, 1024], F32, tag="sc", name="sc")
                sc3 = sc.rearrange("p (h q) -> p h q", h=2)
                nc.tensor.matmul(sc3[:, 0, off:512], KT[0:64, jj],
                                 QT[0:64, 512 * c + off:512 * (c + 1)],
                                 start=True, stop=True)
                nc.tensor.matmul(sc3[:, 1, off:512], KT[64:128, jj],
                                 QT[64:128, 512 * c + off:512 * (c + 1)],
                                 start=True, stop=True)
                if USE_FP8_AV and t < 0:
                    m = j % 2
                    if m == 0:
                        ptp_cur = pt_pool.tile([128, 2, 1024], FP8,
                                               tag="pt8", name="ptp")
                    pt3 = ptp_cur[:, m].rearrange("p (h q) -> p h q", h=2)
                    nc.scalar.activation(out=pt3[:, :, :], in_=sc3[:, :, :],
                                         func=EXP, bias=nb_sb)
                    if m == 1:
                        pending.append(("pair", j // 2, ptp_cur, 0))
                else:
                    ptd = pt_pool.tile([128, 1024], CDT, tag="ptd", name="ptd")
                    pt3 = ptd.rearrange("p (h q) -> p h q", h=2)
                    nc.scalar.activation(out=pt3[:, :, off:512],
                                         in_=sc3[:, :, off:512], func=EXP,
                                         bias=nb_sb)
                    if t >= 0:
                        # diagonal band: post-exp multiplicative triu 0/1 mask
                        # on bf16 SBUF (in-place, same pattern as v1)
                        nc.vector.tensor_mul(pt3[:, :, off:off + 128],
                                             pt3[:, :, off:off + 128], tri_sb)
                    kind = "diag" if USE_FP8_AV else "single"
                    pending.append((kind, j, ptd, off))
                while len(pending) > 2:
                    kind, a, b, o2 = pending.pop(0)
                    if kind == "pair":
                        emit_av_pair(a, b, o2)
                    elif kind == "diag":
                        emit_av_diag(a, b, o2)
                    else:
                        emit_av_single(a, b, o2)
                pop_filler(2 if c <= 2 else 1)
            for kind, a, b, o2 in pending:
                if kind == "pair":
                    emit_av_pair(a, b, o2)
                elif kind == "diag":
                    emit_av_diag(a, b, o2)
                else:
                    emit_av_single(a, b, o2)

            # any undrained QKV units for c+1 must run before its attention
            while filler_q:
                filler_q.popleft()()

            # softmax denominators (row 64) -> rescale OT
            recs, bcs = [], []
            for ot in (ot0, ot1):
                rec = rec_pool.tile([1, 512], F32, tag="rec", name="rec")
                nc.vector._custom_dve(RECIPROCAL_APPROX_FAST, out=rec,
                                      in0=ot[64:65, :],
                                      s0=RECIP_APPROX_FAST_CONSTS["s0"],
                                      s1=RECIP_APPROX_FAST_CONSTS["s1"],
                                      imm2=RECIP_APPROX_FAST_CONSTS["imm2"])
                recs.append(rec)
            for h in (0, 1):
                bc = bc_pool.tile([64, 512], F32, tag="bc", name="bc")
                nc.gpsimd.partition_broadcast(bc, recs[h])
                bcs.append(bc)
            for h, ot in ((0, ot0), (1, ot1)):
                nc.vector.tensor_mul(OT[64 * h:64 * (h + 1), cc], ot[0:64, :],
                                     bcs[h])
            push_proj(c)

        while filler_p:
            filler_p.popleft()()


_NC_CACHE = {}


def _get_nc():
    if "nc" not in _NC_CACHE:
        _NC_CACHE["nc"] = _build_nc()
    return _NC_CACHE["nc"]


def make_in_maps(x, Wq, bq, Wk, bk, Wv, bv, Wo, bo):
    import ml_dtypes
    cdt = ml_dtypes.bfloat16
    x = np.asarray(x, np.float32).reshape(S, D)
    xT = np.ascontiguousarray(x.T).astype(cdt)
    scale = 1.0 / np.sqrt(HD)
    # additive causal mask for the 128x128 diagonal band (transposed scores:
    # pt[sk, q] masked iff sk > q within the band)
    r = np.arange(128)
    tri2 = np.where(r[:, None] > r[None, :], 0.0, 1.0).astype(np.float32)
    tri2 = np.ascontiguousarray(
        np.broadcast_to(tri2[:, None, :], (128, 2, 128))).astype(cdt)
    ident = np.eye(128, dtype=np.float32).astype(cdt)
    in_maps = []
    for c in range(NCORES):
        cs = slice(CPC * c, CPC * (c + 1))
        # w3[p, proj, k, c2] = W[128k+p, c2] for the three projections
        w3 = np.stack([np.asarray(Wq)[:, cs] * scale,
                       np.asarray(Wk)[:, cs],
                       np.asarray(Wv)[:, cs]], axis=1)  # [D, 3, 128]
        w3 = np.ascontiguousarray(
            w3.reshape(ND, 128, 3, CPC).transpose(1, 2, 0, 3)).astype(cdt)
        in_maps.append({
            "xT": xT,
            "w3": w3,
            "wo": np.ascontiguousarray(np.asarray(Wo)[cs, :]).astype(cdt),
            "bq": np.ascontiguousarray(np.asarray(bq)[cs] * scale).reshape(CPC, 1),
            "bk": np.ascontiguousarray(np.asarray(bk)[cs]).reshape(CPC, 1),
            "bv": np.ascontiguousarray(np.asarray(bv)[cs]).reshape(CPC, 1),
            "tri2": tri2,
            "ident": ident,
        })
    return in_maps


def kernel(x, Wq, bq, Wk, bk, Wv, bv, Wo, bo, _run_kwargs=None):
    nc = _get_nc()
    in_maps = make_in_maps(x, Wq, bq, Wk, bk, Wv, bv, Wo, bo)
    res = run_bass_kernel_spmd(nc, in_maps, list(range(NCORES)),
                               **(_run_kwargs or {}))
    acc = np.zeros((S, D), np.float64)
    for c in range(NCORES):
        acc += np.asarray(res.results[c]["out"], np.float64)
    full = (acc + np.asarray(bo, np.float64)).astype(np.float32)
    if _run_kwargs is not None:
        _NC_CACHE["last_results"] = res
    return full.reshape(1, S, D)
